# revision 23
# baseline (speedup 1.0000x reference)
"""Trainium2 Bass kernel for nn_AttnHGCN (2-hop attention GNN + user aggregation).

Strategy (8 NeuronCores, SPMD):
- Nodes partitioned 12500/core by head; edges assigned to the core owning their
  head, grouped per 128-node window. Entity table replicated via on-device
  bf16 AllGather (Shared output) each hop.
- Math: softmax max/denominator cancel under the trailing l2norm, so each hop is
  ent' = l2norm(segment_sum(exp(exp(dot)) * te)), dot = ent[h].(rel[r]*ent[t]).
- Window-major layout: padded local row id = slot*WPC + window, so each core's
  own window rows live SBUF-resident as [128, WPC*D] across the whole stage
  (no per-window ew loads), and l2norm output DMAs are contiguous.
- Edges within each window sorted by tail; chunk k of a window = k-th tail
  quantile. Batches = same-rank chunks across an 8-window block; one
  dma_gather (int16 idx, per-batch base) fetches all tail rows of a batch.
  Gathers round-robin across 4 SWDGE queues so transfers overlap.
- Per chunk: head rows via one-hot fp8 matmul from the resident window tile;
  dot via two DVE passes + reduce; aggregation via m2^T @ (w8*te) accumulated
  in per-window PSUM tiles; l2norm batched over 8 windows (one [128,8D] tile,
  broadcast-scaled in one DVE op per sink).
- hop0 tail rows are static: pre-gathered on host (bf16), no device gathers.
"""
import numpy as np
import ml_dtypes

import concourse.bass as bass
import concourse.bacc as bacc
import concourse.tile as tile
import concourse.mybir as mybir
from concourse import bass_utils

F32 = mybir.dt.float32
BF16 = mybir.dt.bfloat16
FP8 = mybir.dt.float8e4
I16 = mybir.dt.int16

N_CORES = 8
N_NODES = 100000
N_USERS = 100000
D = 128
NPC = N_NODES // N_CORES          # nodes per core
WPC = (NPC + 127) // 128          # windows per core (98)
WB = WPC * 128                    # padded rows per core (12544)
NTAB = N_CORES * WB               # full padded table rows (100352)
BLK = 8                           # windows per block (= live PSUM agg tiles)
NB = 8                            # l2norm batch (windows per sqrt batch)
GSPAN = 32768                     # int16 gather range per batch
NQ = 4                            # SWDGE queues for gathers

_f8 = lambda x: np.ascontiguousarray(x).astype(ml_dtypes.float8_e4m3)
_bf = lambda x: np.ascontiguousarray(x).astype(ml_dtypes.bfloat16)


def _balance_perm(deg, n_heavy=0):
    """LPT bin-packing of nodes into WPC windows of <=128 slots minimizing the
    max window edge-sum; windows relabeled by descending load so heavy windows
    share indices across cores. Returns perm (w*128+slot -> old_local or -1)."""
    import heapq
    order = np.argsort(-deg, kind="stable")
    nb = WPC - (n_heavy or 0)
    if n_heavy:
        heavy, order = order[:128 * n_heavy], order[128 * n_heavy:]
    heap = [(0, 0, w) for w in range(nb)]
    heapq.heapify(heap)
    members = [[] for _ in range(nb)]
    loads = np.zeros(nb, np.int64)
    stashed = []
    for n in order:
        while True:
            load, cnt, w = heapq.heappop(heap)
            if cnt < 128:
                break
            stashed.append((load, cnt, w))
        members[w].append(n)
        loads[w] = load + int(deg[n])
        heapq.heappush(heap, (loads[w], cnt + 1, w))
        for it in stashed:
            heapq.heappush(heap, it)
        stashed.clear()
    out = np.full(WPC * 128, -1, np.int64)
    for rank, w in enumerate(np.argsort(-loads, kind="stable")):
        vals = members[w]
        out[(rank + (n_heavy or 0)) * 128:(rank + (n_heavy or 0)) * 128 + len(vals)] = vals
    if n_heavy:
        out[:128 * n_heavy] = heavy
    return out


def _wmajor(perm_ws):
    """Convert a (w*128+slot)-indexed per-core perm to window-major
    (slot*WPC+w)-indexed."""
    out = np.full(WB, -1, np.int64)
    for w in range(WPC):
        for_w = perm_ws[w * 128:(w + 1) * 128]
        out[np.arange(128) * WPC + w] = for_w
    return out


def _build_structure(kw):
    """Batch structure shared by all cores: list of (c0, [(cid, w, k, first,
    last), ...]) with chunk ids in emission order. Batch = same-rank chunks of
    one 8-window block."""
    batches = []
    cid = 0
    for b0 in range(0, WPC, BLK):
        wins = list(range(b0, min(b0 + BLK, WPC)))
        kmax = max(int(kw[w]) for w in wins)
        for k in range(kmax):
            mem = [w for w in wins if int(kw[w]) > k]
            if not mem:
                continue
            c0 = cid
            entries = []
            for w in mem:
                entries.append((cid, w, k, k == 0, k == int(kw[w]) - 1))
                cid += 1
            batches.append([c0, entries])
    return batches, cid


def _split_wide_batches(batches, lo, hi):
    """Split batches whose tail span exceeds GSPAN (rare). lo/hi are per-chunk
    [nch] min/max valid tail (or +inf/-inf). Returns new (batches, nch) with
    chunk ids REUSED (ids don't change; only grouping does)."""
    out = []
    for c0, entries in batches:
        cur = []
        cur_lo, cur_hi = np.inf, -np.inf
        for ent in entries:
            cid = ent[0]
            nlo = min(cur_lo, lo[cid])
            nhi = max(cur_hi, hi[cid])
            if cur and nhi - nlo >= GSPAN - 128:
                out.append([cur[0][0], cur])
                cur = [ent]
                cur_lo, cur_hi = lo[cid], hi[cid]
            else:
                cur.append(ent)
                cur_lo, cur_hi = nlo, nhi
        if cur:
            out.append([cur[0][0], cur])
    return out


def _preprocess(item_emb, edge_index, edge_type, inter_edge, inter_edge_w,
                relation_emb):
    head = np.asarray(edge_index[0]).astype(np.int64)
    tail = np.asarray(edge_index[1]).astype(np.int64)
    rtyp = (np.asarray(edge_type).astype(np.int64) - 1).astype(np.int32)
    u_idx = np.asarray(inter_edge[0]).astype(np.int64)
    i_idx = np.asarray(inter_edge[1]).astype(np.int64)
    w_int = np.asarray(inter_edge_w).astype(np.float32)

    # window-major global row id: c*WB + slot*WPC + w
    perm_ent = np.empty(N_CORES * WB, np.int64)
    inv_ent = np.empty(N_NODES, np.int64)
    perm_usr = np.empty(N_CORES * WB, np.int64)
    inv_usr = np.empty(N_USERS, np.int64)
    deg_h = np.bincount(head, minlength=N_NODES)
    deg_u = np.bincount(u_idx, minlength=N_USERS)
    for c in range(N_CORES):
        p = _wmajor(_balance_perm(deg_h[c * NPC:(c + 1) * NPC]))
        perm_ent[c * WB:(c + 1) * WB] = np.where(p >= 0, p + c * NPC, -1)
        valid = p >= 0
        inv_ent[p[valid] + c * NPC] = np.nonzero(valid)[0] + c * WB
        pu = _wmajor(_balance_perm(deg_u[c * NPC:(c + 1) * NPC], n_heavy=2))
        perm_usr[c * WB:(c + 1) * WB] = np.where(pu >= 0, pu + c * NPC, -1)
        validu = pu >= 0
        inv_usr[pu[validu] + c * NPC] = np.nonzero(validu)[0] + c * WB

    head_n = inv_ent[head]
    tail_n = inv_ent[tail]
    u_n = inv_usr[u_idx]
    i_n = inv_ent[i_idx]

    # per-core sorted edge lists; value-aligned chunk cuts shared across cores
    # (<=128 edges per core AND global tail span <= PACK_SPAN per chunk)
    PACK_SPAN = GSPAN - 2048

    def value_cuts(tail_lists):
        """tail_lists: per-core sorted tails of one window. Returns per-core
        cut index lists (same length for all cores)."""
        ptrs = [0] * N_CORES
        cuts = [[0] for _ in range(N_CORES)]
        while any(p < len(t) for p, t in zip(ptrs, tail_lists)):
            v = min(t[p] for p, t in zip(ptrs, tail_lists) if p < len(t))
            cap = min((t[p + 128] for p, t in zip(ptrs, tail_lists)
                       if p + 128 < len(t)), default=np.inf)
            v_next = min(v + PACK_SPAN, max(cap, v + 1))
            for c in range(N_CORES):
                t, p = tail_lists[c], ptrs[c]
                e = min(p + 128, int(np.searchsorted(t, v_next, side="left")))
                e = max(e, p)
                cuts[c].append(e)
                ptrs[c] = e
        return cuts

    def core_sorted(src_global, aux1, aux2):
        cores = []
        for c in range(N_CORES):
            m = (src_global >= c * WB) & (src_global < (c + 1) * WB)
            loc = (src_global[m] - c * WB).astype(np.int64)
            tl = aux1[m]
            ax = aux2[m]
            wins = loc % WPC
            order = np.lexsort((tl, wins))
            loc, tl, ax, wins = loc[order], tl[order], ax[order], wins[order]
            bounds = np.searchsorted(wins, np.arange(WPC + 1))
            cores.append([loc, tl, ax, bounds, None])
        # shared value cuts per window
        kw = np.ones(WPC, np.int64)
        for c in range(N_CORES):
            cores[c][4] = [None] * WPC
        for w in range(WPC):
            tls = [cores[c][1][cores[c][3][w]:cores[c][3][w + 1]] for c in range(N_CORES)]
            cuts = value_cuts(tls)
            kw[w] = max(1, len(cuts[0]) - 1)
            for c in range(N_CORES):
                cores[c][4][w] = cuts[c]
        return cores, kw

    core_h, kw_h = core_sorted(head_n, tail_n, rtyp.astype(np.float64))
    core_i, kw_i = core_sorted(u_n, i_n, w_int.astype(np.float64))

    bat_h, nch_h = _build_structure(kw_h)
    bat_i, nch_i = _build_structure(kw_i)

    def fill_grid(cores, batches, nch):
        """Per-core [nch,128] arrays: local head slot hlp (-1 pad), tail row,
        rel/w."""
        res = []
        lo = np.full(nch, np.inf)
        hi = np.full(nch, -np.inf)
        for c in range(N_CORES):
            loc, tl, aux, bounds, cuts = cores[c]
            hlp = np.full((nch, 128), -1, np.int32)
            tlp = np.zeros((nch, 128), np.int64)
            axp = np.zeros((nch, 128), np.float64)
            for c0, entries in batches:
                for (cid, w, k, first, last) in entries:
                    cc = cuts[w]
                    if k >= len(cc) - 1:
                        continue
                    s = int(bounds[w]) + int(cc[k])
                    e = int(bounds[w]) + int(cc[k + 1])
                    if e <= s:
                        continue
                    n = e - s
                    hlp[cid, :n] = loc[s:e] // WPC
                    tlp[cid, :n] = tl[s:e]
                    axp[cid, :n] = aux[s:e]
                    lo[cid] = min(lo[cid], float(tl[s]))
                    hi[cid] = max(hi[cid], float(tl[e - 1]))
            res.append((hlp, tlp, axp))
        return res, lo, hi

    grid_h, lo_h, hi_h = fill_grid(core_h, bat_h, nch_h)
    bat_h = _split_wide_batches(bat_h, lo_h, hi_h)
    grid_i, lo_i, hi_i = fill_grid(core_i, bat_i, nch_i)
    bat_i = _split_wide_batches(bat_i, lo_i, hi_i)

    def batch_bases(batches, lo, hi):
        bases = []
        for c0, entries in batches:
            blo = min((lo[e[0]] for e in entries), default=np.inf)
            bhi = max((hi[e[0]] for e in entries), default=-np.inf)
            if not np.isfinite(blo):
                bases.append(0)
                continue
            assert bhi - blo < GSPAN, f"batch span {bhi-blo} >= {GSPAN}"
            bases.append(int(min(blo, NTAB - 1)))
        return bases

    bases_h = batch_bases(bat_h, lo_h, hi_h)
    bases_i = batch_bases(bat_i, lo_i, hi_i)

    item_f = np.asarray(item_emb, dtype=np.float32)
    rel_f = np.asarray(relation_emb, dtype=np.float32)
    reltab_f = np.zeros((16, D), np.float32)
    reltab_f[:rel_f.shape[0]] = rel_f
    reltab_f = _bf(reltab_f).astype(np.float32)

    def masks_packed(hlp, ree_rows=None):
        """Packed [128, nch*512] u8 (as fp8): per chunk 128B m1, 128B m2,
        256B ree (bf16). If ree_rows is None, [128, nch*256] with masks only."""
        nch = hlp.shape[0]
        ch_i, e_i = np.nonzero(hlp >= 0)
        m2 = np.zeros((nch, 128, 128), np.float32)
        m2[ch_i, e_i, hlp[ch_i, e_i]] = 1.0
        cw = 512 if ree_rows is not None else 256
        pk = np.zeros((128, nch, cw), np.uint8)
        # m1[node, ch, e]: partition = node
        pk[:, :, 0:128] = _f8(m2.transpose(2, 0, 1)).view(np.uint8)
        # m2[e, ch, node]: partition = edge slot
        pk[:, :, 128:256] = _f8(m2.transpose(1, 0, 2)).view(np.uint8)
        if ree_rows is not None:
            # ree[e, ch, d] bf16 -> bytes
            pk[:, :, 256:512] = _bf(ree_rows).view(np.uint8)
        return pk.reshape(128, nch * cw).view(ml_dtypes.float8_e4m3), ch_i, e_i

    def wrap_idx(batches, bases, tlp, hlp, nch):
        # [128, nch*8]: the 16-partition wrap replicated 8x (one copy per
        # GPSIMD Q7 core; desc-gen for queue q>0 runs on a different core
        # which reads its own 16-partition replica).
        big = np.zeros((128, nch * 8), np.int16)
        for (c0, entries), base in zip(batches, bases):
            k = len(entries)
            idx = np.zeros((k, 128), np.int64)
            for j, (cid, w, kk, first, last) in enumerate(entries):
                v = hlp[cid] >= 0
                idx[j, v] = tlp[cid, v] - base
            flat = idx.reshape(-1)
            wrapped = flat.reshape(k * 8, 16).T.astype(np.int16)
            for r in range(8):
                big[16 * r:16 * (r + 1), c0 * 8:(c0 + k) * 8] = wrapped
        return big

    per_core = []
    for c in range(N_CORES):
        hlp, tlp, rlp = grid_h[c]
        ch_i, e_i = np.nonzero(hlp >= 0)
        ree = np.zeros((nch_h, 128, D), np.float32)
        ree[ch_i, e_i] = reltab_f[rlp.astype(np.int32)[ch_i, e_i]]
        ree_e = np.ascontiguousarray(ree.transpose(1, 0, 2))  # [e, nch, D]
        mm, ch_i, e_i = masks_packed(hlp, ree_e)

        # hop-0 tails pre-gathered (original node ids via perm)
        h0 = np.zeros((nch_h, 128, D), np.float32)
        orig = perm_ent[tlp[ch_i, e_i]]
        h0[ch_i, e_i] = item_f[orig]
        h0te = _bf(np.ascontiguousarray(h0.transpose(1, 0, 2)).reshape(128, nch_h * D))

        h1idx = wrap_idx(bat_h, bases_h, tlp, hlp, nch_h)

        ulp, ilp, wvp = grid_i[c]
        # inter stage only needs the scatter mask m2 ([e, node] layout)
        uch_i, ue_i = np.nonzero(ulp >= 0)
        im2 = np.zeros((nch_i, 128, 128), np.float32)
        im2[uch_i, ue_i, ulp[uch_i, ue_i]] = 1.0
        imm = _f8(np.ascontiguousarray(im2.transpose(1, 0, 2)).reshape(128, nch_i * 128))
        iw = np.zeros((nch_i, 128), np.float32)
        iw[uch_i, ue_i] = wvp[uch_i, ue_i]
        iwl = _bf(np.ascontiguousarray(iw.T))  # [128, nch_i]
        # packed [idx k*8 | iw k] per batch, int16
        iidx_pure = wrap_idx(bat_i, bases_i, ilp, ulp, nch_i)
        total9 = sum(len(e) * 9 for _, e in bat_i)
        iidx = np.zeros((128, total9), np.int16)
        off = 0
        for (c0, entries) in bat_i:
            k = len(entries)
            iidx[:, off:off + k * 8] = iidx_pure[:, c0 * 8:(c0 + k) * 8]
            iidx[:, off + k * 8:off + k * 9] = iwl[:, c0:c0 + k].view(np.int16)
            off += k * 9

        # window-major entity slice [128, WPC*D]: row l=p*WPC+w at [p, w*D:]
        pe = perm_ent[c * WB:(c + 1) * WB]
        ent_slice = np.zeros((WB, D), np.float32)
        vv = pe >= 0
        ent_slice[vv] = item_f[pe[vv]]
        ent_slice = ent_slice.reshape(128, WPC * D)

        per_core.append(dict(mm=mm, h0te=h0te, h1idx=h1idx,
                             imm=imm, iidx=iidx,
                             ent_slice=_bf(ent_slice)))
    struct = (tuple((c0, tuple(entries)) for c0, entries in bat_h),
              tuple((c0, tuple(entries)) for c0, entries in bat_i),
              nch_h, nch_i, tuple(bases_h), tuple(bases_i))
    return per_core, struct, perm_ent, perm_usr


def _build_program(struct, n_hops):
    bat_h, bat_i, nch_h, nch_i, bases_h, bases_i = struct
    nc = bacc.Bacc("TRN2", target_bir_lowering=False, debug=False,
                   num_devices=N_CORES, num_swdge_queues=NQ,
                   dynamic_dma_scratch_size=32768)
    t_slice = nc.dram_tensor("ent_slice", [128, WPC * D], BF16, kind="ExternalInput")
    t_mm = nc.dram_tensor("mm", [128, nch_h * 512], FP8, kind="ExternalInput")
    t_h0te = nc.dram_tensor("h0te", [128, nch_h * D], BF16, kind="ExternalInput")
    t_hidx = nc.dram_tensor("h1idx", [128, nch_h * 8], I16, kind="ExternalInput")
    t_imm = nc.dram_tensor("imm", [128, nch_i * 128], FP8, kind="ExternalInput")
    total9 = sum(len(e) * 9 for _, e in bat_i)
    ioffs = []
    off = 0
    for (c0, entries) in bat_i:
        ioffs.append(off)
        off += len(entries) * 9
    t_iidx = nc.dram_tensor("iidx", [128, total9], I16, kind="ExternalInput")
    o_ent = nc.dram_tensor("ent_out", [128, WPC * D], F32, kind="ExternalOutput")
    o_usr = nc.dram_tensor("user_out", [128, WPC * D], F32, kind="ExternalOutput")

    MULT = mybir.AluOpType.mult
    BYP = mybir.AluOpType.bypass
    EXP = mybir.ActivationFunctionType.Exp
    SQRT = mybir.ActivationFunctionType.Sqrt
    SQ = mybir.ActivationFunctionType.Square

    with tile.TileContext(nc) as tc:
        with (
            tc.tile_pool(name="sbte", bufs=6) as sbte,
            tc.tile_pool(name="sb3", bufs=3) as sb3,
            tc.tile_pool(name="sbi", bufs=6) as sbi,
            tc.tile_pool(name="sbs", bufs=4) as sbs,
            tc.tile_pool(name="sbew", bufs=2) as sbew,
            tc.tile_pool(name="norm", bufs=3) as nbp,
            tc.tile_pool(name="nsc", bufs=NB + 2) as nsc,
            tc.tile_pool(name="pshe", bufs=2, space="PSUM") as ps_he,
            tc.tile_pool(name="psagg", bufs=2, space="PSUM") as ps_agg,
            tc.tile_pool(name="dram", bufs=1, space="DRAM") as dram,
        ):
            tabs = [None]
            for h in range(1, n_hops + 1):
                tabs.append((dram.tile([128, WPC * D], BF16, tag=f"agin{h}",
                                       name=f"agin{h}"),
                             dram.tile([NTAB, D], BF16, tag=f"agout{h}",
                                       name=f"agout{h}", addr_space="Shared")))

            # resident window-row tiles (current hop input / next hop output)
            ew_tiles = [sbew.tile([128, WPC * D], BF16, tag="ewres", name=f"ew{h}")
                        for h in range(n_hops)]
            nc.sync.dma_start(ew_tiles[0][:], t_slice.ap())

            def stage(h):
                """h in [0, n_hops): KG hop. h == n_hops: inter aggregation."""
                is_hop = h < n_hops
                batches = bat_h if is_hop else bat_i
                bases = bases_h if is_hop else bases_i
                gtab = None
                if h >= 1:
                    gtab = tabs[min(h, n_hops)][1]
                ew_cur = ew_tiles[h] if is_hop else None
                ew_next = ew_tiles[h + 1] if (is_hop and h + 1 < n_hops) else None
                if is_hop:
                    bf_sink = tabs[h + 1][0]
                    f32_sink = o_ent if h == n_hops - 1 else None
                else:
                    bf_sink = None
                    f32_sink = o_usr

                pend = []
                ssqs = None

                def flush(aggt_f):
                    nonlocal ssqs
                    nw = len(pend)
                    if nw == 0:
                        return
                    # windows complete out of order within a block; slot = w%BLK.
                    # a full block occupies slots 0..7; the last block (nw<8)
                    # occupies slots 0..nw-1. Values read straight from PSUM.
                    w0 = (pend[0] // BLK) * BLK
                    assert sorted(w % BLK for w in pend) == list(range(nw))
                    ssq_c = nsc.tile([128, NB], F32, tag="ssqc")
                    nc.vector.tensor_scalar_max(ssq_c[:, :nw], ssqs[:, :nw], 1e-24)
                    nrm = nsc.tile([128, NB], F32, tag="nrm")
                    nc.scalar.activation(nrm[:, :nw], ssq_c[:, :nw], SQRT)
                    inv = nsc.tile([128, NB], F32, tag="inv")
                    nc.vector.reciprocal(inv[:, :nw], nrm[:, :nw])
                    inv_b = inv[:, :nw].rearrange("p (k o) -> p k o", o=1) \
                        .to_broadcast([128, nw, D])
                    src3 = aggt_f[:, 0:nw * D].rearrange("p (k d) -> p k d", d=D)
                    cols = slice(w0 * D, (w0 + nw) * D)
                    if bf_sink is not None:
                        if ew_next is not None:
                            ob = ew_next[:, cols]
                        else:
                            obt = nbp.tile([128, NB * D], BF16, tag="outbf")
                            ob = obt[:, 0:nw * D]
                        nc.vector.tensor_tensor(
                            out=ob.rearrange("p (k d) -> p k d", d=D),
                            in0=src3, in1=inv_b, op=MULT)
                        nc.sync.dma_start(bf_sink[:, cols], ob)
                    if f32_sink is not None:
                        of = nbp.tile([128, NB * D], F32, tag="outf32")
                        nc.vector.tensor_tensor(
                            out=of[:, 0:nw * D].rearrange("p (k d) -> p k d", d=D),
                            in0=src3, in1=inv_b, op=MULT)
                        nc.sync.dma_start(f32_sink.ap()[:, cols], of[:, 0:nw * D])
                    pend.clear()

                aggt = None
                cur_blk = -1
                for bi, (c0, entries) in enumerate(batches):
                    k = len(entries)
                    blk = entries[0][1] // BLK
                    if blk != cur_blk:
                        cur_blk = blk
                        aggt = ps_agg.tile([128, BLK * D], F32, tag="agg",
                                           name="agg")
                        nc.vector.memset(aggt[:], 0.0)
                    cols = slice(c0 * 128, (c0 + k) * 128)
                    kc = slice(0, k * 128)

                    te = sbte.tile([128, BLK * 128], BF16, tag="te")
                    if h == 0:
                        nc.sync.dma_start(te[:, 0:k * D],
                                          t_h0te.ap()[:, c0 * D:(c0 + k) * D])
                    else:
                        idxt = sbi.tile([128, BLK * 9], I16, tag="idx")
                        if is_hop:
                            nc.scalar.dma_start(idxt[:, 0:k * 8],
                                                t_hidx.ap()[:, c0 * 8:(c0 + k) * 8])
                        else:
                            io = ioffs[bi]
                            nc.scalar.dma_start(idxt[:, 0:k * 9],
                                                t_iidx.ap()[:, io:io + k * 9])
                        base = bases[bi]
                        hi = min(base + GSPAN, NTAB)
                        nc.gpsimd.dma_gather(
                            out_ap=te[:, kc].rearrange("p (k d) -> p k d", d=D),
                            in_ap=gtab[base:hi, :],
                            idxs_ap=idxt[:, 0:k * 8],
                            num_idxs=k * 128,
                            num_idxs_reg=k * 128,
                            elem_size=D,
                            queue_num=bi % NQ,
                        )

                    if is_hop:
                        mmt = sb3.tile([128, BLK * 512], FP8, tag="mm")
                        nc.sync.dma_start(mmt[:, 0:k * 512],
                                          t_mm.ap()[:, c0 * 512:(c0 + k) * 512])
                        he = ps_he.tile([128, BLK * 128], F32, tag="he")
                        for j, (cid, w, kk, first, last) in enumerate(entries):
                            sl = slice(j * 128, (j + 1) * 128)
                            nc.tensor.matmul(
                                he[:, sl], mmt[:, j * 512:j * 512 + 128],
                                ew_cur[:, w * D:(w + 1) * D],
                                start=True, stop=True)
                        ret3 = mmt[:].bitcast(BF16)[:, 0:k * 256] \
                            .rearrange("p (k x) -> p k x", x=256)[:, :, 128:256]
                        vt = sb3.tile([128, BLK * 128], BF16, tag="vt")
                        nc.vector.tensor_tensor(
                            out=vt[:, kc].rearrange("p (k d) -> p k d", d=D),
                            in0=te[:, kc].rearrange("p (k d) -> p k d", d=D),
                            in1=ret3, op=MULT)
                        p2 = sb3.tile([128, BLK * 128], BF16, tag="p2")
                        nc.vector.tensor_tensor(out=p2[:, kc], in0=vt[:, kc],
                                                in1=he[:, kc], op=MULT)
                        dots = sbs.tile([128, BLK], F32, tag="dots")
                        nc.vector.tensor_reduce(
                            out=dots[:, :k],
                            in_=p2[:, kc].rearrange("p (k d) -> p k d", d=D),
                            axis=mybir.AxisListType.X, op=mybir.AluOpType.add)
                        e1 = sbs.tile([128, BLK], F32, tag="e1")
                        nc.scalar.activation(e1[:, :k], dots[:, :k], EXP)
                        wsc = sbs.tile([128, BLK], BF16, tag="w8")
                        nc.scalar.activation(wsc[:, :k], e1[:, :k], EXP)
                    else:
                        mmt = sb3.tile([128, BLK * 512], FP8, tag="mm")
                        nc.sync.dma_start(mmt[:, 0:k * 128],
                                          t_imm.ap()[:, c0 * 128:(c0 + k) * 128])
                        wsc = idxt[:, k * 8:k * 9].bitcast(BF16)

                    tew = sb3.tile([128, BLK * 128], BF16, tag="tew")
                    nc.vector.tensor_tensor(
                        out=tew[:, kc].rearrange("p (k d) -> p k d", d=D),
                        in0=te[:, kc].rearrange("p (k d) -> p k d", d=D),
                        in1=wsc[:, :k].rearrange("p (k o) -> p k o", o=1)
                            .to_broadcast([128, k, D]),
                        op=MULT)

                    # interleave window order so consecutive matmuls alternate
                    # PSUM banks (w%BLK*512B: windows 0-3 bank A, 4-7 bank B)
                    order = sorted(range(k), key=lambda j: (entries[j][1] % BLK) % 4 * 2
                                   + (entries[j][1] % BLK) // 4)
                    for j in order:
                        (cid, w, kk, first, last) = entries[j]
                        sl = slice(j * 128, (j + 1) * 128)
                        aggv = aggt[:, (w % BLK) * D:(w % BLK + 1) * D]
                        m2sl = (mmt[:, j * 512 + 128:j * 512 + 256] if is_hop
                                else mmt[:, j * 128:(j + 1) * 128])
                        nc.tensor.matmul(aggv, m2sl, tew[:, sl],
                                         start=False, stop=bool(last),
                                         skip_group_check=True)
                    for j, (cid, w, kk, first, last) in enumerate(entries):
                        if not last:
                            continue
                        aggv = aggt[:, (w % BLK) * D:(w % BLK + 1) * D]
                        if not pend:
                            ssqs = nsc.tile([128, NB], F32, tag="ssqs")
                        i8 = w % BLK
                        scr = nsc.tile([128, D], F32, tag="sqscr")
                        nc.scalar.activation(scr[:], aggv, SQ,
                                             accum_out=ssqs[:, i8:i8 + 1])
                        pend.append(w)
                        if len(pend) == NB:
                            flush(aggt)
                flush(aggt)
                if is_hop:
                    nc.gpsimd.collective_compute(
                        "AllGather", BYP, replica_groups=[list(range(N_CORES))],
                        ins=[tabs[h + 1][0].opt()],
                        outs=[tabs[h + 1][1].opt()],
                    )

            for h in range(n_hops + 1):
                stage(h)
    nc.compile()
    return nc


_CACHE = {}


def kernel(user_emb, item_emb, edge_index, edge_type, inter_edge, inter_edge_w,
           relation_emb, n_hops, _trace=False):
    n_hops = int(n_hops)
    item_emb = np.asarray(item_emb, dtype=np.float32)
    relation_emb = np.asarray(relation_emb, dtype=np.float32)

    per_core, struct, perm_ent, perm_usr = _preprocess(
        item_emb, edge_index, edge_type, inter_edge, inter_edge_w, relation_emb)
    key = (struct, n_hops)
    if key not in _CACHE:
        _CACHE[key] = _build_program(struct, n_hops)
    nc = _CACHE[key]

    in_maps = []
    for c in range(N_CORES):
        pc = per_core[c]
        in_maps.append({
            "ent_slice": pc["ent_slice"],
            "mm": pc["mm"],
            "h0te": pc["h0te"], "h1idx": pc["h1idx"],
            "imm": pc["imm"], "iidx": pc["iidx"],
        })
    import os
    kw = {}
    if _trace and os.environ.get("KERNEL_NTFF_DIR"):
        os.makedirs(os.environ["KERNEL_NTFF_DIR"], exist_ok=True)
        kw["tmpdir"] = os.environ["KERNEL_NTFF_DIR"]
    res = bass_utils.run_bass_kernel_spmd(
        nc, in_maps, core_ids=list(range(N_CORES)), trace=_trace, **kw,
    )
    # outputs are [128, WPC*D] window-major: row l=p*WPC+w -> [p, w*D:(w+1)*D]
    ent_p = np.concatenate(
        [res.results[c]["ent_out"].reshape(WB, D) for c in range(N_CORES)], 0)
    usr_p = np.concatenate(
        [res.results[c]["user_out"].reshape(WB, D) for c in range(N_CORES)], 0)
    vmask = perm_ent >= 0
    ent = np.empty((N_NODES, D), np.float32)
    ent[perm_ent[vmask]] = ent_p[vmask]
    usr = np.empty((N_USERS, D), np.float32)
    vmask_u = perm_usr >= 0
    usr[perm_usr[vmask_u]] = usr_p[vmask_u]
    if _trace:
        kernel._last_exec_ns = res.exec_time_ns
        kernel._last_res = res
    return usr, ent


# revision 26
# speedup vs baseline: 1.0292x; 1.0292x over previous
"""Trainium2 Bass kernel for nn_AttnHGCN (2-hop attention GNN + user aggregation).

Strategy (8 NeuronCores, SPMD):
- Nodes partitioned 12500/core by head; edges assigned to the core owning their
  head, grouped per 128-node window. Entity table replicated via on-device
  bf16 AllGather (Shared output) each hop.
- Math: softmax max/denominator cancel under the trailing l2norm, so each hop is
  ent' = l2norm(segment_sum(exp(exp(dot)) * te)), dot = ent[h].(rel[r]*ent[t]).
- Window-major layout: padded local row id = slot*WPC + window, so each core's
  own window rows live SBUF-resident as [128, WPC*D] across the whole stage
  (no per-window ew loads), and l2norm output DMAs are contiguous.
- Edges within each window sorted by tail; chunk k of a window = k-th tail
  quantile. Batches = same-rank chunks across an 8-window block; one
  dma_gather (int16 idx, per-batch base) fetches all tail rows of a batch.
  Gathers round-robin across 4 SWDGE queues so transfers overlap.
- Per chunk: head rows via one-hot fp8 matmul from the resident window tile;
  dot via two DVE passes + reduce; aggregation via m2^T @ (w8*te) accumulated
  in per-window PSUM tiles; l2norm batched over 8 windows (one [128,8D] tile,
  broadcast-scaled in one DVE op per sink).
- hop0 tail rows are static: pre-gathered on host (bf16), no device gathers.
"""
import numpy as np
import ml_dtypes

import concourse.bass as bass
import concourse.bacc as bacc
import concourse.tile as tile
import concourse.mybir as mybir
from concourse import bass_utils

F32 = mybir.dt.float32
BF16 = mybir.dt.bfloat16
FP8 = mybir.dt.float8e4
I16 = mybir.dt.int16

N_CORES = 8
N_NODES = 100000
N_USERS = 100000
D = 128
NPC = N_NODES // N_CORES          # nodes per core
WPC = (NPC + 127) // 128          # windows per core (98)
WB = WPC * 128                    # padded rows per core (12544)
NTAB = N_CORES * WB               # full padded table rows (100352)
BLK = 8                           # windows per block (= live PSUM agg tiles)
NB = 8                            # l2norm batch (windows per sqrt batch)
GSPAN = 32768                     # int16 gather range per batch
NQ = 4                            # SWDGE queues for gathers

_f8 = lambda x: np.ascontiguousarray(x).astype(ml_dtypes.float8_e4m3)
_bf = lambda x: np.ascontiguousarray(x).astype(ml_dtypes.bfloat16)


def _balance_perm(deg, n_heavy=0):
    """LPT bin-packing of nodes into WPC windows of <=128 slots minimizing the
    max window edge-sum; windows relabeled by descending load so heavy windows
    share indices across cores. Returns perm (w*128+slot -> old_local or -1)."""
    import heapq
    order = np.argsort(-deg, kind="stable")
    nb = WPC - (n_heavy or 0)
    if n_heavy:
        heavy, order = order[:128 * n_heavy], order[128 * n_heavy:]
    heap = [(0, 0, w) for w in range(nb)]
    heapq.heapify(heap)
    members = [[] for _ in range(nb)]
    loads = np.zeros(nb, np.int64)
    stashed = []
    for n in order:
        while True:
            load, cnt, w = heapq.heappop(heap)
            if cnt < 128:
                break
            stashed.append((load, cnt, w))
        members[w].append(n)
        loads[w] = load + int(deg[n])
        heapq.heappush(heap, (loads[w], cnt + 1, w))
        for it in stashed:
            heapq.heappush(heap, it)
        stashed.clear()
    out = np.full(WPC * 128, -1, np.int64)
    for rank, w in enumerate(np.argsort(-loads, kind="stable")):
        vals = members[w]
        out[(rank + (n_heavy or 0)) * 128:(rank + (n_heavy or 0)) * 128 + len(vals)] = vals
    if n_heavy:
        out[:128 * n_heavy] = heavy
    return out


def _wmajor(perm_ws):
    """Convert a (w*128+slot)-indexed per-core perm to window-major
    (slot*WPC+w)-indexed."""
    out = np.full(WB, -1, np.int64)
    for w in range(WPC):
        for_w = perm_ws[w * 128:(w + 1) * 128]
        out[np.arange(128) * WPC + w] = for_w
    return out


def _build_structure(kw):
    """Batch structure shared by all cores: list of (c0, [(cid, w, k, first,
    last), ...]) with chunk ids in emission order. Batch = same-rank chunks of
    one 8-window block."""
    batches = []
    cid = 0
    for b0 in range(0, WPC, BLK):
        wins = list(range(b0, min(b0 + BLK, WPC)))
        kmax = max(int(kw[w]) for w in wins)
        for k in range(kmax):
            mem = [w for w in wins if int(kw[w]) > k]
            if not mem:
                continue
            c0 = cid
            entries = []
            for w in mem:
                entries.append((cid, w, k, k == 0, k == int(kw[w]) - 1))
                cid += 1
            batches.append([c0, entries])
    return batches, cid


def _split_wide_batches(batches, lo, hi):
    """Split batches whose tail span exceeds GSPAN (rare). lo/hi are per-chunk
    [nch] min/max valid tail (or +inf/-inf). Returns new (batches, nch) with
    chunk ids REUSED (ids don't change; only grouping does)."""
    out = []
    for c0, entries in batches:
        cur = []
        cur_lo, cur_hi = np.inf, -np.inf
        for ent in entries:
            cid = ent[0]
            nlo = min(cur_lo, lo[cid])
            nhi = max(cur_hi, hi[cid])
            if cur and nhi - nlo >= GSPAN - 128:
                out.append([cur[0][0], cur])
                cur = [ent]
                cur_lo, cur_hi = lo[cid], hi[cid]
            else:
                cur.append(ent)
                cur_lo, cur_hi = nlo, nhi
        if cur:
            out.append([cur[0][0], cur])
    return out


def _preprocess(item_emb, edge_index, edge_type, inter_edge, inter_edge_w,
                relation_emb):
    head = np.asarray(edge_index[0]).astype(np.int64)
    tail = np.asarray(edge_index[1]).astype(np.int64)
    rtyp = (np.asarray(edge_type).astype(np.int64) - 1).astype(np.int32)
    u_idx = np.asarray(inter_edge[0]).astype(np.int64)
    i_idx = np.asarray(inter_edge[1]).astype(np.int64)
    w_int = np.asarray(inter_edge_w).astype(np.float32)

    # window-major global row id: c*WB + slot*WPC + w
    perm_ent = np.empty(N_CORES * WB, np.int64)
    inv_ent = np.empty(N_NODES, np.int64)
    perm_usr = np.empty(N_CORES * WB, np.int64)
    inv_usr = np.empty(N_USERS, np.int64)
    deg_h = np.bincount(head, minlength=N_NODES)
    deg_u = np.bincount(u_idx, minlength=N_USERS)
    for c in range(N_CORES):
        p = _wmajor(_balance_perm(deg_h[c * NPC:(c + 1) * NPC]))
        perm_ent[c * WB:(c + 1) * WB] = np.where(p >= 0, p + c * NPC, -1)
        valid = p >= 0
        inv_ent[p[valid] + c * NPC] = np.nonzero(valid)[0] + c * WB
        pu = _wmajor(_balance_perm(deg_u[c * NPC:(c + 1) * NPC], n_heavy=2))
        perm_usr[c * WB:(c + 1) * WB] = np.where(pu >= 0, pu + c * NPC, -1)
        validu = pu >= 0
        inv_usr[pu[validu] + c * NPC] = np.nonzero(validu)[0] + c * WB

    head_n = inv_ent[head]
    tail_n = inv_ent[tail]
    u_n = inv_usr[u_idx]
    i_n = inv_ent[i_idx]

    # per-core sorted edge lists; value-aligned chunk cuts shared across cores
    # (<=128 edges per core AND global tail span <= PACK_SPAN per chunk)
    PACK_SPAN = GSPAN - 2048

    def value_cuts(tail_lists):
        """tail_lists: per-core sorted tails of one window. Returns per-core
        cut index lists (same length for all cores)."""
        ptrs = [0] * N_CORES
        cuts = [[0] for _ in range(N_CORES)]
        while any(p < len(t) for p, t in zip(ptrs, tail_lists)):
            v = min(t[p] for p, t in zip(ptrs, tail_lists) if p < len(t))
            cap = min((t[p + 128] for p, t in zip(ptrs, tail_lists)
                       if p + 128 < len(t)), default=np.inf)
            v_next = min(v + PACK_SPAN, max(cap, v + 1))
            for c in range(N_CORES):
                t, p = tail_lists[c], ptrs[c]
                e = min(p + 128, int(np.searchsorted(t, v_next, side="left")))
                e = max(e, p)
                cuts[c].append(e)
                ptrs[c] = e
        return cuts

    def core_sorted(src_global, aux1, aux2):
        cores = []
        for c in range(N_CORES):
            m = (src_global >= c * WB) & (src_global < (c + 1) * WB)
            loc = (src_global[m] - c * WB).astype(np.int64)
            tl = aux1[m]
            ax = aux2[m]
            wins = loc % WPC
            order = np.lexsort((tl, wins))
            loc, tl, ax, wins = loc[order], tl[order], ax[order], wins[order]
            bounds = np.searchsorted(wins, np.arange(WPC + 1))
            cores.append([loc, tl, ax, bounds, None])
        # shared value cuts per window
        kw = np.ones(WPC, np.int64)
        for c in range(N_CORES):
            cores[c][4] = [None] * WPC
        for w in range(WPC):
            tls = [cores[c][1][cores[c][3][w]:cores[c][3][w + 1]] for c in range(N_CORES)]
            cuts = value_cuts(tls)
            kw[w] = max(1, len(cuts[0]) - 1)
            for c in range(N_CORES):
                cores[c][4][w] = cuts[c]
        return cores, kw

    core_h, kw_h = core_sorted(head_n, tail_n, rtyp.astype(np.float64))
    core_i, kw_i = core_sorted(u_n, i_n, w_int.astype(np.float64))

    bat_h, nch_h = _build_structure(kw_h)
    bat_i, nch_i = _build_structure(kw_i)

    def fill_grid(cores, batches, nch):
        """Per-core [nch,128] arrays: local head slot hlp (-1 pad), tail row,
        rel/w."""
        res = []
        lo = np.full(nch, np.inf)
        hi = np.full(nch, -np.inf)
        for c in range(N_CORES):
            loc, tl, aux, bounds, cuts = cores[c]
            hlp = np.full((nch, 128), -1, np.int32)
            tlp = np.zeros((nch, 128), np.int64)
            axp = np.zeros((nch, 128), np.float64)
            for c0, entries in batches:
                for (cid, w, k, first, last) in entries:
                    cc = cuts[w]
                    if k >= len(cc) - 1:
                        continue
                    s = int(bounds[w]) + int(cc[k])
                    e = int(bounds[w]) + int(cc[k + 1])
                    if e <= s:
                        continue
                    n = e - s
                    hlp[cid, :n] = loc[s:e] // WPC
                    tlp[cid, :n] = tl[s:e]
                    axp[cid, :n] = aux[s:e]
                    lo[cid] = min(lo[cid], float(tl[s]))
                    hi[cid] = max(hi[cid], float(tl[e - 1]))
            res.append((hlp, tlp, axp))
        return res, lo, hi

    grid_h, lo_h, hi_h = fill_grid(core_h, bat_h, nch_h)
    bat_h = _split_wide_batches(bat_h, lo_h, hi_h)
    grid_i, lo_i, hi_i = fill_grid(core_i, bat_i, nch_i)
    bat_i = _split_wide_batches(bat_i, lo_i, hi_i)

    def batch_bases(batches, lo, hi):
        bases = []
        for c0, entries in batches:
            blo = min((lo[e[0]] for e in entries), default=np.inf)
            bhi = max((hi[e[0]] for e in entries), default=-np.inf)
            if not np.isfinite(blo):
                bases.append(0)
                continue
            assert bhi - blo < GSPAN, f"batch span {bhi-blo} >= {GSPAN}"
            bases.append(int(min(blo, NTAB - 1)))
        return bases

    bases_h = batch_bases(bat_h, lo_h, hi_h)
    bases_i = batch_bases(bat_i, lo_i, hi_i)

    item_f = np.asarray(item_emb, dtype=np.float32)
    rel_f = np.asarray(relation_emb, dtype=np.float32)
    reltab_f = np.zeros((16, D), np.float32)
    reltab_f[:rel_f.shape[0]] = rel_f
    reltab_f = _bf(reltab_f).astype(np.float32)

    def masks_packed(hlp, ree_rows=None):
        """Packed [128, nch*512] u8 (as fp8): per chunk 128B m1, 128B m2,
        256B ree (bf16). If ree_rows is None, [128, nch*256] with masks only."""
        nch = hlp.shape[0]
        ch_i, e_i = np.nonzero(hlp >= 0)
        m2 = np.zeros((nch, 128, 128), np.float32)
        m2[ch_i, e_i, hlp[ch_i, e_i]] = 1.0
        cw = 512 if ree_rows is not None else 256
        pk = np.zeros((128, nch, cw), np.uint8)
        # m1[node, ch, e]: partition = node
        pk[:, :, 0:128] = _f8(m2.transpose(2, 0, 1)).view(np.uint8)
        # m2[e, ch, node]: partition = edge slot
        pk[:, :, 128:256] = _f8(m2.transpose(1, 0, 2)).view(np.uint8)
        if ree_rows is not None:
            # ree[e, ch, d] bf16 -> bytes
            pk[:, :, 256:512] = _bf(ree_rows).view(np.uint8)
        return pk.reshape(128, nch * cw).view(ml_dtypes.float8_e4m3), ch_i, e_i

    def wrap_idx(batches, bases, tlp, hlp, nch):
        # [128, nch*8]: the 16-partition wrap replicated 8x (one copy per
        # GPSIMD Q7 core; desc-gen for queue q>0 runs on a different core
        # which reads its own 16-partition replica).
        big = np.zeros((128, nch * 8), np.int16)
        for (c0, entries), base in zip(batches, bases):
            k = len(entries)
            idx = np.zeros((k, 128), np.int64)
            for j, (cid, w, kk, first, last) in enumerate(entries):
                v = hlp[cid] >= 0
                idx[j, v] = tlp[cid, v] - base
            flat = idx.reshape(-1)
            wrapped = flat.reshape(k * 8, 16).T.astype(np.int16)
            for r in range(8):
                big[16 * r:16 * (r + 1), c0 * 8:(c0 + k) * 8] = wrapped
        return big

    per_core = []
    for c in range(N_CORES):
        hlp, tlp, rlp = grid_h[c]
        ch_i, e_i = np.nonzero(hlp >= 0)
        ree = np.zeros((nch_h, 128, D), np.float32)
        ree[ch_i, e_i] = reltab_f[rlp.astype(np.int32)[ch_i, e_i]]
        ree_e = np.ascontiguousarray(ree.transpose(1, 0, 2))  # [e, nch, D]
        mm, ch_i, e_i = masks_packed(hlp, ree_e)

        # hop-0 tails pre-gathered (original node ids via perm)
        h0 = np.zeros((nch_h, 128, D), np.float32)
        orig = perm_ent[tlp[ch_i, e_i]]
        h0[ch_i, e_i] = item_f[orig]
        h0te = _bf(np.ascontiguousarray(h0.transpose(1, 0, 2)).reshape(128, nch_h * D))

        h1idx = wrap_idx(bat_h, bases_h, tlp, hlp, nch_h)

        ulp, ilp, wvp = grid_i[c]
        # inter stage only needs the scatter mask m2 ([e, node] layout)
        uch_i, ue_i = np.nonzero(ulp >= 0)
        im2 = np.zeros((nch_i, 128, 128), np.float32)
        im2[uch_i, ue_i, ulp[uch_i, ue_i]] = 1.0
        imm = _f8(np.ascontiguousarray(im2.transpose(1, 0, 2)).reshape(128, nch_i * 128))
        iw = np.zeros((nch_i, 128), np.float32)
        iw[uch_i, ue_i] = wvp[uch_i, ue_i]
        iwl = _bf(np.ascontiguousarray(iw.T))  # [128, nch_i]
        # packed [idx k*8 | iw k] per batch, int16
        iidx_pure = wrap_idx(bat_i, bases_i, ilp, ulp, nch_i)
        total9 = sum(len(e) * 9 for _, e in bat_i)
        iidx = np.zeros((128, total9), np.int16)
        off = 0
        for (c0, entries) in bat_i:
            k = len(entries)
            iidx[:, off:off + k * 8] = iidx_pure[:, c0 * 8:(c0 + k) * 8]
            iidx[:, off + k * 8:off + k * 9] = iwl[:, c0:c0 + k].view(np.int16)
            off += k * 9

        # window-major entity slice [128, WPC*D]: row l=p*WPC+w at [p, w*D:]
        pe = perm_ent[c * WB:(c + 1) * WB]
        ent_slice = np.zeros((WB, D), np.float32)
        vv = pe >= 0
        ent_slice[vv] = item_f[pe[vv]]
        ent_slice = ent_slice.reshape(128, WPC * D)

        per_core.append(dict(mm=mm, h0te=h0te, h1idx=h1idx,
                             imm=imm, iidx=iidx,
                             ent_slice=_bf(ent_slice)))
    struct = (tuple((c0, tuple(entries)) for c0, entries in bat_h),
              tuple((c0, tuple(entries)) for c0, entries in bat_i),
              nch_h, nch_i, tuple(bases_h), tuple(bases_i))
    return per_core, struct, perm_ent, perm_usr


def _build_program(struct, n_hops):
    bat_h, bat_i, nch_h, nch_i, bases_h, bases_i = struct
    nc = bacc.Bacc("TRN2", target_bir_lowering=False, debug=False,
                   num_devices=N_CORES, num_swdge_queues=NQ,
                   dynamic_dma_scratch_size=32768)
    t_slice = nc.dram_tensor("ent_slice", [128, WPC * D], BF16, kind="ExternalInput")
    t_mm = nc.dram_tensor("mm", [128, nch_h * 512], FP8, kind="ExternalInput")
    t_h0te = nc.dram_tensor("h0te", [128, nch_h * D], BF16, kind="ExternalInput")
    t_hidx = nc.dram_tensor("h1idx", [128, nch_h * 8], I16, kind="ExternalInput")
    t_imm = nc.dram_tensor("imm", [128, nch_i * 128], FP8, kind="ExternalInput")
    total9 = sum(len(e) * 9 for _, e in bat_i)
    ioffs = []
    off = 0
    for (c0, entries) in bat_i:
        ioffs.append(off)
        off += len(entries) * 9
    t_iidx = nc.dram_tensor("iidx", [128, total9], I16, kind="ExternalInput")
    o_ent = nc.dram_tensor("ent_out", [128, WPC * D], F32, kind="ExternalOutput")
    o_usr = nc.dram_tensor("user_out", [128, WPC * D], F32, kind="ExternalOutput")

    MULT = mybir.AluOpType.mult
    BYP = mybir.AluOpType.bypass
    EXP = mybir.ActivationFunctionType.Exp
    SQRT = mybir.ActivationFunctionType.Sqrt
    SQ = mybir.ActivationFunctionType.Square

    with tile.TileContext(nc) as tc:
        with (
            tc.tile_pool(name="sbte", bufs=6) as sbte,
            tc.tile_pool(name="sb3", bufs=3) as sb3,
            tc.tile_pool(name="sbi", bufs=6) as sbi,
            tc.tile_pool(name="sbs", bufs=4) as sbs,
            tc.tile_pool(name="sbew", bufs=2) as sbew,
            tc.tile_pool(name="norm", bufs=3) as nbp,
            tc.tile_pool(name="nsc", bufs=NB + 2) as nsc,
            tc.tile_pool(name="pshe", bufs=2, space="PSUM") as ps_he,
            tc.tile_pool(name="psagg", bufs=2, space="PSUM") as ps_agg,
            tc.tile_pool(name="dram", bufs=1, space="DRAM") as dram,
        ):
            tabs = [None]
            for h in range(1, n_hops + 1):
                tabs.append((dram.tile([128, WPC * D], BF16, tag=f"agin{h}",
                                       name=f"agin{h}"),
                             dram.tile([NTAB, D], BF16, tag=f"agout{h}",
                                       name=f"agout{h}", addr_space="Shared")))

            # resident window-row tiles (current hop input / next hop output)
            ew_tiles = [sbew.tile([128, WPC * D], BF16, tag="ewres", name=f"ew{h}")
                        for h in range(n_hops)]
            nc.sync.dma_start(ew_tiles[0][:], t_slice.ap())

            def stage(h):
                """h in [0, n_hops): KG hop. h == n_hops: inter aggregation."""
                is_hop = h < n_hops
                batches = bat_h if is_hop else bat_i
                bases = bases_h if is_hop else bases_i
                gtab = None
                if h >= 1:
                    gtab = tabs[min(h, n_hops)][1]
                ew_cur = ew_tiles[h] if is_hop else None
                ew_next = ew_tiles[h + 1] if (is_hop and h + 1 < n_hops) else None
                if is_hop:
                    bf_sink = tabs[h + 1][0]
                    f32_sink = o_ent if h == n_hops - 1 else None
                else:
                    bf_sink = None
                    f32_sink = o_usr

                pend = []
                ent8 = None
                ssqs = None

                def flush(aggt_f):
                    nonlocal ent8, ssqs
                    nw = len(pend)
                    if nw == 0:
                        return
                    # windows complete out of order within a block; slot = w%BLK.
                    # a full block occupies slots 0..7; the last block (nw<8)
                    # occupies slots 0..nw-1.
                    w0 = (pend[0] // BLK) * BLK
                    assert sorted(w % BLK for w in pend) == list(range(nw))
                    ssq_c = nsc.tile([128, NB], F32, tag="ssqc")
                    nc.vector.tensor_scalar_max(ssq_c[:, :nw], ssqs[:, :nw], 1e-24)
                    nrm = nsc.tile([128, NB], F32, tag="nrm")
                    nc.scalar.activation(nrm[:, :nw], ssq_c[:, :nw], SQRT)
                    inv = nsc.tile([128, NB], F32, tag="inv")
                    nc.vector.reciprocal(inv[:, :nw], nrm[:, :nw])
                    inv_b = inv[:, :nw].rearrange("p (k o) -> p k o", o=1) \
                        .to_broadcast([128, nw, D])
                    src3 = ent8[:, 0:nw * D].rearrange("p (k d) -> p k d", d=D)
                    cols = slice(w0 * D, (w0 + nw) * D)
                    if bf_sink is not None:
                        if ew_next is not None:
                            ob = ew_next[:, cols]
                        else:
                            obt = nbp.tile([128, NB * D], BF16, tag="outbf")
                            ob = obt[:, 0:nw * D]
                        nc.vector.tensor_tensor(
                            out=ob.rearrange("p (k d) -> p k d", d=D),
                            in0=src3, in1=inv_b, op=MULT)
                        nc.sync.dma_start(bf_sink[:, cols], ob)
                    if f32_sink is not None:
                        of = nbp.tile([128, NB * D], F32, tag="outf32")
                        nc.vector.tensor_tensor(
                            out=of[:, 0:nw * D].rearrange("p (k d) -> p k d", d=D),
                            in0=src3, in1=inv_b, op=MULT)
                        nc.sync.dma_start(f32_sink.ap()[:, cols], of[:, 0:nw * D])
                    pend.clear()

                aggt = None
                cur_blk = -1
                for bi, (c0, entries) in enumerate(batches):
                    k = len(entries)
                    blk = entries[0][1] // BLK
                    if blk != cur_blk:
                        cur_blk = blk
                        aggt = ps_agg.tile([128, BLK * D], F32, tag="agg",
                                           name="agg")
                        nc.vector.memset(aggt[:], 0.0)
                    cols = slice(c0 * 128, (c0 + k) * 128)
                    kc = slice(0, k * 128)

                    te = sbte.tile([128, BLK * 128], BF16, tag="te")
                    if h == 0:
                        nc.sync.dma_start(te[:, 0:k * D],
                                          t_h0te.ap()[:, c0 * D:(c0 + k) * D])
                    else:
                        idxt = sbi.tile([128, BLK * 9], I16, tag="idx")
                        if is_hop:
                            nc.scalar.dma_start(idxt[:, 0:k * 8],
                                                t_hidx.ap()[:, c0 * 8:(c0 + k) * 8])
                        else:
                            io = ioffs[bi]
                            nc.scalar.dma_start(idxt[:, 0:k * 9],
                                                t_iidx.ap()[:, io:io + k * 9])
                        base = bases[bi]
                        hi = min(base + GSPAN, NTAB)
                        nc.gpsimd.dma_gather(
                            out_ap=te[:, kc].rearrange("p (k d) -> p k d", d=D),
                            in_ap=gtab[base:hi, :],
                            idxs_ap=idxt[:, 0:k * 8],
                            num_idxs=k * 128,
                            num_idxs_reg=k * 128,
                            elem_size=D,
                            queue_num=bi % NQ,
                        )

                    if is_hop:
                        mmt = sb3.tile([128, BLK * 512], FP8, tag="mm")
                        nc.sync.dma_start(mmt[:, 0:k * 512],
                                          t_mm.ap()[:, c0 * 512:(c0 + k) * 512])
                        he = ps_he.tile([128, BLK * 128], F32, tag="he")
                        for j, (cid, w, kk, first, last) in enumerate(entries):
                            sl = slice(j * 128, (j + 1) * 128)
                            nc.tensor.matmul(
                                he[:, sl], mmt[:, j * 512:j * 512 + 128],
                                ew_cur[:, w * D:(w + 1) * D],
                                start=True, stop=True)
                        ret3 = mmt[:].bitcast(BF16)[:, 0:k * 256] \
                            .rearrange("p (k x) -> p k x", x=256)[:, :, 128:256]
                        vt = sb3.tile([128, BLK * 128], BF16, tag="vt")
                        nc.vector.tensor_tensor(
                            out=vt[:, kc].rearrange("p (k d) -> p k d", d=D),
                            in0=te[:, kc].rearrange("p (k d) -> p k d", d=D),
                            in1=ret3, op=MULT)
                        p2 = sb3.tile([128, BLK * 128], BF16, tag="p2")
                        nc.vector.tensor_tensor(out=p2[:, kc], in0=vt[:, kc],
                                                in1=he[:, kc], op=MULT)
                        dots = sbs.tile([128, BLK], F32, tag="dots")
                        nc.vector.tensor_reduce(
                            out=dots[:, :k],
                            in_=p2[:, kc].rearrange("p (k d) -> p k d", d=D),
                            axis=mybir.AxisListType.X, op=mybir.AluOpType.add)
                        e1 = sbs.tile([128, BLK], F32, tag="e1")
                        nc.scalar.activation(e1[:, :k], dots[:, :k], EXP)
                        wsc = sbs.tile([128, BLK], BF16, tag="w8")
                        nc.scalar.activation(wsc[:, :k], e1[:, :k], EXP)
                    else:
                        mmt = sb3.tile([128, BLK * 512], FP8, tag="mm")
                        nc.sync.dma_start(mmt[:, 0:k * 128],
                                          t_imm.ap()[:, c0 * 128:(c0 + k) * 128])
                        wsc = idxt[:, k * 8:k * 9].bitcast(BF16)

                    tew = sb3.tile([128, BLK * 128], BF16, tag="tew")
                    nc.vector.tensor_tensor(
                        out=tew[:, kc].rearrange("p (k d) -> p k d", d=D),
                        in0=te[:, kc].rearrange("p (k d) -> p k d", d=D),
                        in1=wsc[:, :k].rearrange("p (k o) -> p k o", o=1)
                            .to_broadcast([128, k, D]),
                        op=MULT)

                    # interleave window order so consecutive matmuls alternate
                    # PSUM banks (w%BLK*512B: windows 0-3 bank A, 4-7 bank B)
                    order = sorted(range(k), key=lambda j: (entries[j][1] % BLK) % 4 * 2
                                   + (entries[j][1] % BLK) // 4)
                    for j in order:
                        (cid, w, kk, first, last) = entries[j]
                        sl = slice(j * 128, (j + 1) * 128)
                        aggv = aggt[:, (w % BLK) * D:(w % BLK + 1) * D]
                        m2sl = (mmt[:, j * 512 + 128:j * 512 + 256] if is_hop
                                else mmt[:, j * 128:(j + 1) * 128])
                        nc.tensor.matmul(aggv, m2sl, tew[:, sl],
                                         start=False, stop=bool(last),
                                         skip_group_check=True)
                    for j, (cid, w, kk, first, last) in enumerate(entries):
                        if not last:
                            continue
                        aggv = aggt[:, (w % BLK) * D:(w % BLK + 1) * D]
                        if not pend:
                            ent8 = nbp.tile([128, NB * D], F32, tag="ent8")
                            ssqs = nsc.tile([128, NB], F32, tag="ssqs")
                        i8 = w % BLK
                        dst = ent8[:, i8 * D:(i8 + 1) * D]
                        nc.scalar.copy(dst, aggv)
                        scr = nsc.tile([128, D], F32, tag="sqscr")
                        nc.scalar.activation(scr[:], dst, SQ,
                                             accum_out=ssqs[:, i8:i8 + 1])
                        pend.append(w)
                        if len(pend) == NB:
                            flush(aggt)
                flush(aggt)
                if is_hop:
                    nc.gpsimd.collective_compute(
                        "AllGather", BYP, replica_groups=[list(range(N_CORES))],
                        ins=[tabs[h + 1][0].opt()],
                        outs=[tabs[h + 1][1].opt()],
                    )

            for h in range(n_hops + 1):
                stage(h)
    nc.compile()
    return nc


_CACHE = {}


def kernel(user_emb, item_emb, edge_index, edge_type, inter_edge, inter_edge_w,
           relation_emb, n_hops, _trace=False):
    n_hops = int(n_hops)
    item_emb = np.asarray(item_emb, dtype=np.float32)
    relation_emb = np.asarray(relation_emb, dtype=np.float32)

    per_core, struct, perm_ent, perm_usr = _preprocess(
        item_emb, edge_index, edge_type, inter_edge, inter_edge_w, relation_emb)
    key = (struct, n_hops)
    if key not in _CACHE:
        _CACHE[key] = _build_program(struct, n_hops)
    nc = _CACHE[key]

    in_maps = []
    for c in range(N_CORES):
        pc = per_core[c]
        in_maps.append({
            "ent_slice": pc["ent_slice"],
            "mm": pc["mm"],
            "h0te": pc["h0te"], "h1idx": pc["h1idx"],
            "imm": pc["imm"], "iidx": pc["iidx"],
        })
    import os
    kw = {}
    if _trace and os.environ.get("KERNEL_NTFF_DIR"):
        os.makedirs(os.environ["KERNEL_NTFF_DIR"], exist_ok=True)
        kw["tmpdir"] = os.environ["KERNEL_NTFF_DIR"]
    res = bass_utils.run_bass_kernel_spmd(
        nc, in_maps, core_ids=list(range(N_CORES)), trace=_trace, **kw,
    )
    # outputs are [128, WPC*D] window-major: row l=p*WPC+w -> [p, w*D:(w+1)*D]
    ent_p = np.concatenate(
        [res.results[c]["ent_out"].reshape(WB, D) for c in range(N_CORES)], 0)
    usr_p = np.concatenate(
        [res.results[c]["user_out"].reshape(WB, D) for c in range(N_CORES)], 0)
    vmask = perm_ent >= 0
    ent = np.empty((N_NODES, D), np.float32)
    ent[perm_ent[vmask]] = ent_p[vmask]
    usr = np.empty((N_USERS, D), np.float32)
    vmask_u = perm_usr >= 0
    usr[perm_usr[vmask_u]] = usr_p[vmask_u]
    if _trace:
        kernel._last_exec_ns = res.exec_time_ns
        kernel._last_res = res
    return usr, ent


# revision 40
# speedup vs baseline: 1.4359x; 1.3951x over previous
"""Trainium2 Bass kernel for nn_AttnHGCN (2-hop attention GNN + user aggregation).

Strategy (8 NeuronCores, SPMD):
- Nodes partitioned 12500/core by head; edges assigned to the core owning their
  head, grouped per 128-node window. Entity table replicated via on-device
  bf16 AllGather (Shared output) each hop.
- Math: softmax max/denominator cancel under the trailing l2norm, so each hop is
  ent' = l2norm(segment_sum(exp(exp(dot)) * te)), dot = ent[h].(rel[r]*ent[t]).
- Window-major layout: padded local row id = slot*WPC + window, so each core's
  own window rows live SBUF-resident as [128, WPC*D] across the whole stage
  (no per-window ew loads), and l2norm output DMAs are contiguous.
- Edges within each window sorted by tail; chunk k of a window = k-th tail
  quantile. Batches = same-rank chunks across an 8-window block; one
  dma_gather (int16 idx, per-batch base) fetches all tail rows of a batch.
  Gathers round-robin across 4 SWDGE queues so transfers overlap.
- Per chunk: head rows via one-hot fp8 matmul from the resident window tile;
  dot via two DVE passes + reduce; aggregation via m2^T @ (w8*te) accumulated
  in per-window PSUM tiles; l2norm batched over 8 windows (one [128,8D] tile,
  broadcast-scaled in one DVE op per sink).
- hop0 tail rows are static: pre-gathered on host (bf16), no device gathers.
"""
import numpy as np
import ml_dtypes

import concourse.bass as bass
import concourse.bacc as bacc
import concourse.tile as tile
import concourse.mybir as mybir
from concourse import bass_utils

F32 = mybir.dt.float32
BF16 = mybir.dt.bfloat16
FP8 = mybir.dt.float8e4
I16 = mybir.dt.int16

N_CORES = 8
N_NODES = 100000
N_USERS = 100000
D = 128
NPC = N_NODES // N_CORES          # nodes per core
WPC = (NPC + 127) // 128          # windows per core (98)
WB = WPC * 128                    # padded rows per core (12544)
NTAB = N_CORES * WB               # full padded table rows (100352)
BLK = 8                           # windows per block (= live PSUM agg tiles)
NB = 8                            # l2norm batch (windows per sqrt batch)
GSPAN = 32768                     # int16 gather range per batch
NQ = 4                            # SWDGE queues for gathers
# AllGather pieces (window ranges, BLK-aligned) fired as their windows
# complete, overlapping the collective with compute. Table rows are
# piece-major: row = rowbase[p] + (c*128+slot)*nw_p + (w - w_p0).
PIECES = [(0, 48), (48, 80), (80, 96), (96, WPC)]
ROWBASE = [0]
for _w0, _w1 in PIECES:
    ROWBASE.append(ROWBASE[-1] + N_CORES * 128 * (_w1 - _w0))


def _piece_of(w):
    for i, (a, b) in enumerate(PIECES):
        if a <= w < b:
            return i
    raise ValueError(w)

_f8 = lambda x: np.ascontiguousarray(x).astype(ml_dtypes.float8_e4m3)
_bf = lambda x: np.ascontiguousarray(x).astype(ml_dtypes.bfloat16)


def _balance_perm(deg, n_heavy=0):
    """LPT bin-packing of nodes into WPC windows of <=128 slots minimizing the
    max window edge-sum; windows relabeled by descending load so heavy windows
    share indices across cores. Returns perm (w*128+slot -> old_local or -1)."""
    import heapq
    order = np.argsort(-deg, kind="stable")
    nb = WPC - (n_heavy or 0)
    if n_heavy:
        heavy, order = order[:128 * n_heavy], order[128 * n_heavy:]
    heap = [(0, 0, w) for w in range(nb)]
    heapq.heapify(heap)
    members = [[] for _ in range(nb)]
    loads = np.zeros(nb, np.int64)
    stashed = []
    for n in order:
        while True:
            load, cnt, w = heapq.heappop(heap)
            if cnt < 128:
                break
            stashed.append((load, cnt, w))
        members[w].append(n)
        loads[w] = load + int(deg[n])
        heapq.heappush(heap, (loads[w], cnt + 1, w))
        for it in stashed:
            heapq.heappush(heap, it)
        stashed.clear()
    out = np.full(WPC * 128, -1, np.int64)
    for rank, w in enumerate(np.argsort(-loads, kind="stable")):
        vals = members[w]
        out[(rank + (n_heavy or 0)) * 128:(rank + (n_heavy or 0)) * 128 + len(vals)] = vals
    if n_heavy:
        out[:128 * n_heavy] = heavy
    return out


def _wmajor(perm_ws):
    """Convert a (w*128+slot)-indexed per-core perm to window-major
    (slot*WPC+w)-indexed."""
    out = np.full(WB, -1, np.int64)
    for w in range(WPC):
        for_w = perm_ws[w * 128:(w + 1) * 128]
        out[np.arange(128) * WPC + w] = for_w
    return out


def _build_structure(kw):
    """Batch structure shared by all cores: list of (c0, [(cid, w, k, first,
    last), ...]) with chunk ids in emission order. Batch = same-rank chunks of
    one 8-window block."""
    batches = []
    cid = 0
    for b0 in range(0, WPC, BLK):
        wins = list(range(b0, min(b0 + BLK, WPC)))
        kmax = max(int(kw[w]) for w in wins)
        for k in range(kmax):
            mem = [w for w in wins if int(kw[w]) > k]
            if not mem:
                continue
            c0 = cid
            entries = []
            for w in mem:
                entries.append((cid, w, k, k == 0, k == int(kw[w]) - 1))
                cid += 1
            batches.append([c0, entries])
    return batches, cid


def _split_wide_batches(batches, lo, hi):
    """Split batches whose tail span exceeds GSPAN (rare). lo/hi are per-chunk
    [nch] min/max valid tail (or +inf/-inf). Returns new (batches, nch) with
    chunk ids REUSED (ids don't change; only grouping does)."""
    out = []
    for c0, entries in batches:
        cur = []
        cur_lo, cur_hi = np.inf, -np.inf
        for ent in entries:
            cid = ent[0]
            nlo = min(cur_lo, lo[cid])
            nhi = max(cur_hi, hi[cid])
            if cur and nhi - nlo >= GSPAN - 128:
                out.append([cur[0][0], cur])
                cur = [ent]
                cur_lo, cur_hi = lo[cid], hi[cid]
            else:
                cur.append(ent)
                cur_lo, cur_hi = nlo, nhi
        if cur:
            out.append([cur[0][0], cur])
    return out


def _preprocess(item_emb, edge_index, edge_type, inter_edge, inter_edge_w,
                relation_emb):
    head = np.asarray(edge_index[0]).astype(np.int64)
    tail = np.asarray(edge_index[1]).astype(np.int64)
    rtyp = (np.asarray(edge_type).astype(np.int64) - 1).astype(np.int32)
    u_idx = np.asarray(inter_edge[0]).astype(np.int64)
    i_idx = np.asarray(inter_edge[1]).astype(np.int64)
    w_int = np.asarray(inter_edge_w).astype(np.float32)

    # window-major global row id: c*WB + slot*WPC + w
    perm_ent = np.empty(N_CORES * WB, np.int64)
    inv_ent = np.empty(N_NODES, np.int64)
    perm_usr = np.empty(N_CORES * WB, np.int64)
    inv_usr = np.empty(N_USERS, np.int64)
    deg_h = np.bincount(head, minlength=N_NODES)
    deg_u = np.bincount(u_idx, minlength=N_USERS)
    for c in range(N_CORES):
        p = _wmajor(_balance_perm(deg_h[c * NPC:(c + 1) * NPC]))
        perm_ent[c * WB:(c + 1) * WB] = np.where(p >= 0, p + c * NPC, -1)
        valid = p >= 0
        inv_ent[p[valid] + c * NPC] = np.nonzero(valid)[0] + c * WB
        pu = _wmajor(_balance_perm(deg_u[c * NPC:(c + 1) * NPC], n_heavy=2))
        perm_usr[c * WB:(c + 1) * WB] = np.where(pu >= 0, pu + c * NPC, -1)
        validu = pu >= 0
        inv_usr[pu[validu] + c * NPC] = np.nonzero(validu)[0] + c * WB

    # table-row numbering for gathered (tail-side) ids: piece-major
    lnew = np.arange(WB)
    slot_l, w_l = lnew // WPC, lnew % WPC
    piece_l = np.zeros(WB, np.int64)
    nw_l = np.zeros(WB, np.int64)
    wp0_l = np.zeros(WB, np.int64)
    rb_l = np.zeros(WB, np.int64)
    for i, (a, b) in enumerate(PIECES):
        m = (w_l >= a) & (w_l < b)
        piece_l[m] = i
        nw_l[m] = b - a
        wp0_l[m] = a
        rb_l[m] = ROWBASE[i]
    row_of_lnew = np.empty((N_CORES, WB), np.int64)
    for c in range(N_CORES):
        row_of_lnew[c] = rb_l + (c * 128 + slot_l) * nw_l + (w_l - wp0_l)
    inv_tab = np.empty(N_NODES, np.int64)
    tab_node = np.full(NTAB, -1, np.int64)
    for c in range(N_CORES):
        pe_c = perm_ent[c * WB:(c + 1) * WB]
        v = pe_c >= 0
        inv_tab[pe_c[v]] = row_of_lnew[c][v]
        tab_node[row_of_lnew[c]] = pe_c

    head_n = inv_ent[head]
    tail_n = inv_tab[tail]
    u_n = inv_usr[u_idx]
    i_n = inv_tab[i_idx]

    # per-core sorted edge lists; value-aligned chunk cuts shared across cores
    # (<=128 edges per core AND global tail span <= PACK_SPAN per chunk)
    PACK_SPAN = GSPAN - 2048

    def value_cuts(tail_lists):
        """tail_lists: per-core sorted tails of one window. Returns per-core
        cut index lists (same length for all cores)."""
        ptrs = [0] * N_CORES
        cuts = [[0] for _ in range(N_CORES)]
        while any(p < len(t) for p, t in zip(ptrs, tail_lists)):
            v = min(t[p] for p, t in zip(ptrs, tail_lists) if p < len(t))
            cap = min((t[p + 128] for p, t in zip(ptrs, tail_lists)
                       if p + 128 < len(t)), default=np.inf)
            v_next = min(v + PACK_SPAN, max(cap, v + 1))
            for c in range(N_CORES):
                t, p = tail_lists[c], ptrs[c]
                e = min(p + 128, int(np.searchsorted(t, v_next, side="left")))
                e = max(e, p)
                cuts[c].append(e)
                ptrs[c] = e
        return cuts

    def core_sorted(src_global, aux1, aux2):
        cores = []
        for c in range(N_CORES):
            m = (src_global >= c * WB) & (src_global < (c + 1) * WB)
            loc = (src_global[m] - c * WB).astype(np.int64)
            tl = aux1[m]
            ax = aux2[m]
            wins = loc % WPC
            order = np.lexsort((tl, wins))
            loc, tl, ax, wins = loc[order], tl[order], ax[order], wins[order]
            bounds = np.searchsorted(wins, np.arange(WPC + 1))
            cores.append([loc, tl, ax, bounds, None])
        # shared value cuts per window
        kw = np.ones(WPC, np.int64)
        for c in range(N_CORES):
            cores[c][4] = [None] * WPC
        for w in range(WPC):
            tls = [cores[c][1][cores[c][3][w]:cores[c][3][w + 1]] for c in range(N_CORES)]
            cuts = value_cuts(tls)
            kw[w] = max(1, len(cuts[0]) - 1)
            for c in range(N_CORES):
                cores[c][4][w] = cuts[c]
        return cores, kw

    core_h, kw_h = core_sorted(head_n, tail_n, rtyp.astype(np.float64))
    core_i, kw_i = core_sorted(u_n, i_n, w_int.astype(np.float64))

    bat_h, nch_h = _build_structure(kw_h)
    bat_i, nch_i = _build_structure(kw_i)

    def fill_grid(cores, batches, nch):
        """Per-core [nch,128] arrays: local head slot hlp (-1 pad), tail row,
        rel/w."""
        res = []
        lo = np.full(nch, np.inf)
        hi = np.full(nch, -np.inf)
        for c in range(N_CORES):
            loc, tl, aux, bounds, cuts = cores[c]
            hlp = np.full((nch, 128), -1, np.int32)
            tlp = np.zeros((nch, 128), np.int64)
            axp = np.zeros((nch, 128), np.float64)
            for c0, entries in batches:
                for (cid, w, k, first, last) in entries:
                    cc = cuts[w]
                    if k >= len(cc) - 1:
                        continue
                    s = int(bounds[w]) + int(cc[k])
                    e = int(bounds[w]) + int(cc[k + 1])
                    if e <= s:
                        continue
                    n = e - s
                    hlp[cid, :n] = loc[s:e] // WPC
                    tlp[cid, :n] = tl[s:e]
                    axp[cid, :n] = aux[s:e]
                    lo[cid] = min(lo[cid], float(tl[s]))
                    hi[cid] = max(hi[cid], float(tl[e - 1]))
            res.append((hlp, tlp, axp))
        return res, lo, hi

    grid_h, lo_h, hi_h = fill_grid(core_h, bat_h, nch_h)
    bat_h = _split_wide_batches(bat_h, lo_h, hi_h)
    grid_i, lo_i, hi_i = fill_grid(core_i, bat_i, nch_i)
    bat_i = _split_wide_batches(bat_i, lo_i, hi_i)

    def batch_bases(batches, lo, hi):
        bases = []
        for c0, entries in batches:
            blo = min((lo[e[0]] for e in entries), default=np.inf)
            bhi = max((hi[e[0]] for e in entries), default=-np.inf)
            if not np.isfinite(blo):
                bases.append(0)
                continue
            assert bhi - blo < GSPAN, f"batch span {bhi-blo} >= {GSPAN}"
            bases.append(int(min(blo, NTAB - 1)))
        return bases

    bases_h = batch_bases(bat_h, lo_h, hi_h)
    bases_i = batch_bases(bat_i, lo_i, hi_i)

    item_f = np.asarray(item_emb, dtype=np.float32)
    rel_f = np.asarray(relation_emb, dtype=np.float32)
    reltab_f = np.zeros((16, D), np.float32)
    reltab_f[:rel_f.shape[0]] = rel_f
    reltab_f = _bf(reltab_f).astype(np.float32)

    def masks_packed(hlp, ree_rows, batches):
        """[128, nch*512] u8 (as fp8), batch-major: per batch, first k*256B of
        masks (per chunk: 128B m1 | 128B m2), then k*256B of ree (bf16)."""
        nch = hlp.shape[0]
        ch_i, e_i = np.nonzero(hlp >= 0)
        m2 = np.zeros((nch, 128, 128), np.float32)
        m2[ch_i, e_i, hlp[ch_i, e_i]] = 1.0
        m1b = _f8(m2.transpose(2, 0, 1)).view(np.uint8)   # [node, nch, 128]
        m2b = _f8(m2.transpose(1, 0, 2)).view(np.uint8)   # [e, nch, 128]
        reeb = _bf(ree_rows).view(np.uint8)               # [e, nch, 256]
        pk = np.zeros((128, nch * 512), np.uint8)
        for (c0, entries) in batches:
            k = len(entries)
            base = c0 * 512
            mb = pk[:, base:base + k * 256].reshape(128, k, 256)
            mb[:, :, 0:128] = m1b[:, c0:c0 + k, :]
            mb[:, :, 128:256] = m2b[:, c0:c0 + k, :]
            pk[:, base + k * 256:base + k * 512] = \
                reeb[:, c0:c0 + k, :].reshape(128, k * 256)
        return pk.view(ml_dtypes.float8_e4m3), ch_i, e_i

    def wrap_idx(batches, bases, tlp, hlp, nch):
        # [128, nch*8]: the 16-partition wrap replicated 8x (one copy per
        # GPSIMD Q7 core; desc-gen for queue q>0 runs on a different core
        # which reads its own 16-partition replica).
        big = np.zeros((128, nch * 8), np.int16)
        for (c0, entries), base in zip(batches, bases):
            k = len(entries)
            idx = np.zeros((k, 128), np.int64)
            for j, (cid, w, kk, first, last) in enumerate(entries):
                v = hlp[cid] >= 0
                idx[j, v] = tlp[cid, v] - base
            flat = idx.reshape(-1)
            wrapped = flat.reshape(k * 8, 16).T.astype(np.int16)
            for r in range(8):
                big[16 * r:16 * (r + 1), c0 * 8:(c0 + k) * 8] = wrapped
        return big

    per_core = []
    for c in range(N_CORES):
        hlp, tlp, rlp = grid_h[c]
        ch_i, e_i = np.nonzero(hlp >= 0)
        ree = np.zeros((nch_h, 128, D), np.float32)
        ree[ch_i, e_i] = reltab_f[rlp.astype(np.int32)[ch_i, e_i]]
        ree_e = np.ascontiguousarray(ree.transpose(1, 0, 2))  # [e, nch, D]
        mm, ch_i, e_i = masks_packed(hlp, ree_e, bat_h)

        # hop-0 tails pre-gathered (original node ids via table-row map)
        h0 = np.zeros((nch_h, 128, D), np.float32)
        orig = tab_node[tlp[ch_i, e_i]]
        h0[ch_i, e_i] = item_f[orig]
        h0te = _bf(np.ascontiguousarray(h0.transpose(1, 0, 2)).reshape(128, nch_h * D))

        h1idx = wrap_idx(bat_h, bases_h, tlp, hlp, nch_h)

        ulp, ilp, wvp = grid_i[c]
        # inter stage only needs the scatter mask m2 ([e, node] layout)
        uch_i, ue_i = np.nonzero(ulp >= 0)
        im2 = np.zeros((nch_i, 128, 128), np.float32)
        im2[uch_i, ue_i, ulp[uch_i, ue_i]] = 1.0
        imm = _f8(np.ascontiguousarray(im2.transpose(1, 0, 2)).reshape(128, nch_i * 128))
        iw = np.zeros((nch_i, 128), np.float32)
        iw[uch_i, ue_i] = wvp[uch_i, ue_i]
        iwl = _bf(np.ascontiguousarray(iw.T))  # [128, nch_i]
        # packed [idx k*8 | iw k] per batch, int16
        iidx_pure = wrap_idx(bat_i, bases_i, ilp, ulp, nch_i)
        total9 = sum(len(e) * 9 for _, e in bat_i)
        iidx = np.zeros((128, total9), np.int16)
        off = 0
        for (c0, entries) in bat_i:
            k = len(entries)
            iidx[:, off:off + k * 8] = iidx_pure[:, c0 * 8:(c0 + k) * 8]
            iidx[:, off + k * 8:off + k * 9] = iwl[:, c0:c0 + k].view(np.int16)
            off += k * 9

        # window-major entity slice [128, WPC*D]: row l=p*WPC+w at [p, w*D:]
        pe = perm_ent[c * WB:(c + 1) * WB]
        ent_slice = np.zeros((WB, D), np.float32)
        vv = pe >= 0
        ent_slice[vv] = item_f[pe[vv]]
        ent_slice = ent_slice.reshape(128, WPC * D)

        per_core.append(dict(mm=mm, h0te=h0te, h1idx=h1idx,
                             imm=imm, iidx=iidx,
                             ent_slice=_bf(ent_slice)))
    struct = (tuple((c0, tuple(entries)) for c0, entries in bat_h),
              tuple((c0, tuple(entries)) for c0, entries in bat_i),
              nch_h, nch_i, tuple(bases_h), tuple(bases_i))
    return per_core, struct, perm_ent, perm_usr


def _build_program(struct, n_hops):
    bat_h, bat_i, nch_h, nch_i, bases_h, bases_i = struct
    nc = bacc.Bacc("TRN2", target_bir_lowering=False, debug=False,
                   num_devices=N_CORES, num_swdge_queues=NQ)
    t_slice = nc.dram_tensor("ent_slice", [128, WPC * D], BF16, kind="ExternalInput")
    t_mm = nc.dram_tensor("mm", [128, nch_h * 512], FP8, kind="ExternalInput")
    t_h0te = nc.dram_tensor("h0te", [128, nch_h * D], BF16, kind="ExternalInput")
    t_hidx = nc.dram_tensor("h1idx", [128, nch_h * 8], I16, kind="ExternalInput")
    t_imm = nc.dram_tensor("imm", [128, nch_i * 128], FP8, kind="ExternalInput")
    total9 = sum(len(e) * 9 for _, e in bat_i)
    ioffs = []
    off = 0
    for (c0, entries) in bat_i:
        ioffs.append(off)
        off += len(entries) * 9
    t_iidx = nc.dram_tensor("iidx", [128, total9], I16, kind="ExternalInput")
    o_ent = nc.dram_tensor("ent_out", [128, WPC * D], F32, kind="ExternalOutput")
    o_usr = nc.dram_tensor("user_out", [128, WPC * D], F32, kind="ExternalOutput")

    MULT = mybir.AluOpType.mult
    BYP = mybir.AluOpType.bypass
    EXP = mybir.ActivationFunctionType.Exp
    SQRT = mybir.ActivationFunctionType.Sqrt
    SQ = mybir.ActivationFunctionType.Square

    with tile.TileContext(nc) as tc:
        with (
            tc.tile_pool(name="sbte", bufs=6) as sbte,
            tc.tile_pool(name="sb3", bufs=3) as sb3,
            tc.tile_pool(name="sbi", bufs=6) as sbi,
            tc.tile_pool(name="sbs", bufs=4) as sbs,
            tc.tile_pool(name="sbew", bufs=2) as sbew,
            tc.tile_pool(name="norm", bufs=3) as nbp,
            tc.tile_pool(name="nsc", bufs=NB + 2) as nsc,
            tc.tile_pool(name="pshe", bufs=2, space="PSUM") as ps_he,
            tc.tile_pool(name="psagg", bufs=2, space="PSUM") as ps_agg,
            tc.tile_pool(name="dram", bufs=1, space="DRAM") as dram,
        ):
            tabs = [None]
            for h in range(1, n_hops + 1):
                agins = [dram.tile([128, (b - a) * D], BF16, tag=f"agin{h}_{i}",
                                   name=f"agin{h}_{i}")
                         for i, (a, b) in enumerate(PIECES)]
                tabs.append((agins,
                             dram.tile([NTAB, D], BF16, tag=f"agout{h}",
                                       name=f"agout{h}", addr_space="Shared")))

            # resident window-row tiles (current hop input / next hop output)
            ew_tiles = [sbew.tile([128, WPC * D], BF16, tag="ewres", name=f"ew{h}")
                        for h in range(n_hops)]
            nc.sync.dma_start(ew_tiles[0][:], t_slice.ap())
            # whole-stage gather index preloads (keeps per-batch scalar queue
            # free: a per-batch idx DMA would chain behind the previous
            # batch's exps and serialize the gathers)
            idx_h = sbew.tile([128, nch_h * 8], I16, tag="idxh", name="idxh")
            nc.scalar.dma_start(idx_h[:], t_hidx.ap())
            idx_i = sbew.tile([128, total9], I16, tag="idxi", name="idxi")
            nc.scalar.dma_start(idx_i[:], t_iidx.ap())

            def stage(h):
                """h in [0, n_hops): KG hop. h == n_hops: inter aggregation."""
                is_hop = h < n_hops
                batches = bat_h if is_hop else bat_i
                bases = bases_h if is_hop else bases_i
                gtab = None
                if h >= 1:
                    gtab = tabs[min(h, n_hops)][1]
                ew_cur = ew_tiles[h] if is_hop else None
                ew_next = ew_tiles[h + 1] if (is_hop and h + 1 < n_hops) else None
                if is_hop:
                    bf_sinks = tabs[h + 1][0]   # per-piece agin tiles
                    f32_sink = o_ent if h == n_hops - 1 else None
                else:
                    bf_sinks = None
                    f32_sink = o_usr

                pend = []
                ent8 = None
                ssqs = None

                def flush(aggt_f):
                    nonlocal ent8, ssqs
                    nw = len(pend)
                    if nw == 0:
                        return
                    # windows complete out of order within a block; slot = w%BLK.
                    # a full block occupies slots 0..7; the last block (nw<8)
                    # occupies slots 0..nw-1.
                    w0 = (pend[0] // BLK) * BLK
                    assert sorted(w % BLK for w in pend) == list(range(nw))
                    ssq_c = nsc.tile([128, NB], F32, tag="ssqc")
                    nc.vector.tensor_scalar_max(ssq_c[:, :nw], ssqs[:, :nw], 1e-24)
                    nrm = nsc.tile([128, NB], F32, tag="nrm")
                    nc.scalar.activation(nrm[:, :nw], ssq_c[:, :nw], SQRT)
                    inv = nsc.tile([128, NB], F32, tag="inv")
                    nc.vector.reciprocal(inv[:, :nw], nrm[:, :nw])
                    inv_b = inv[:, :nw].rearrange("p (k o) -> p k o", o=1) \
                        .to_broadcast([128, nw, D])
                    src3 = ent8[:, 0:nw * D].rearrange("p (k d) -> p k d", d=D)
                    cols = slice(w0 * D, (w0 + nw) * D)
                    if bf_sinks is not None:
                        if ew_next is not None:
                            ob = ew_next[:, cols]
                        else:
                            obt = nbp.tile([128, NB * D], BF16, tag="outbf")
                            ob = obt[:, 0:nw * D]
                        nc.vector.tensor_tensor(
                            out=ob.rearrange("p (k d) -> p k d", d=D),
                            in0=src3, in1=inv_b, op=MULT)
                        pi = _piece_of(w0)
                        wp0, wp1 = PIECES[pi]
                        nc.sync.dma_start(
                            bf_sinks[pi][:, (w0 - wp0) * D:(w0 - wp0 + nw) * D], ob)
                        if w0 + nw == wp1:
                            # piece complete on this core: fire its AllGather
                            nwp = wp1 - wp0
                            rb = ROWBASE[pi]
                            nc.gpsimd.collective_compute(
                                "AllGather", BYP,
                                replica_groups=[list(range(N_CORES))],
                                ins=[bf_sinks[pi][:]],
                                outs=[tabs[h + 1][1][rb:rb + N_CORES * 128 * nwp, :]],
                            )
                    if f32_sink is not None:
                        of = nbp.tile([128, NB * D], F32, tag="outf32")
                        nc.vector.tensor_tensor(
                            out=of[:, 0:nw * D].rearrange("p (k d) -> p k d", d=D),
                            in0=src3, in1=inv_b, op=MULT)
                        nc.sync.dma_start(f32_sink.ap()[:, cols], of[:, 0:nw * D])
                    pend.clear()

                aggt = None
                cur_blk = -1
                for bi, (c0, entries) in enumerate(batches):
                    k = len(entries)
                    blk = entries[0][1] // BLK
                    if blk != cur_blk:
                        cur_blk = blk
                        aggt = ps_agg.tile([128, BLK * D], F32, tag="agg",
                                           name="agg")
                        nc.vector.memset(aggt[:], 0.0)
                    cols = slice(c0 * 128, (c0 + k) * 128)
                    kc = slice(0, k * 128)

                    te = sbte.tile([128, BLK * 128], BF16, tag="te")
                    if h == 0:
                        nc.sync.dma_start(te[:, 0:k * D],
                                          t_h0te.ap()[:, c0 * D:(c0 + k) * D])
                    else:
                        if is_hop:
                            idx_ap = idx_h[:, c0 * 8:(c0 + k) * 8]
                        else:
                            io = ioffs[bi]
                            idx_ap = idx_i[:, io:io + k * 8]
                        base = bases[bi]
                        hi = min(base + GSPAN, NTAB)
                        nc.gpsimd.dma_gather(
                            out_ap=te[:, kc].rearrange("p (k d) -> p k d", d=D),
                            in_ap=gtab[base:hi, :],
                            idxs_ap=idx_ap,
                            num_idxs=k * 128,
                            num_idxs_reg=k * 128,
                            elem_size=D,
                            queue_num=bi % NQ,
                        )

                    if is_hop:
                        mmt = sb3.tile([128, BLK * 512], FP8, tag="mm")
                        nc.sync.dma_start(mmt[:, 0:k * 256],
                                          t_mm.ap()[:, c0 * 512:c0 * 512 + k * 256])
                        nc.sync.dma_start(
                            mmt[:, k * 256:k * 512],
                            t_mm.ap()[:, c0 * 512 + k * 256:(c0 + k) * 512])
                        he = ps_he.tile([128, BLK * 128], F32, tag="he")
                        for j, (cid, w, kk, first, last) in enumerate(entries):
                            sl = slice(j * 128, (j + 1) * 128)
                            nc.tensor.matmul(
                                he[:, sl], mmt[:, j * 256:j * 256 + 128],
                                ew_cur[:, w * D:(w + 1) * D],
                                start=True, stop=True)
                        ret = mmt[:].bitcast(BF16)[:, k * 128:k * 256]
                        vt = sb3.tile([128, BLK * 128], BF16, tag="vt")
                        nc.vector.tensor_tensor(out=vt[:, kc], in0=te[:, kc],
                                                in1=ret, op=MULT)
                        p2 = sb3.tile([128, BLK * 128], BF16, tag="p2")
                        nc.vector.tensor_tensor(out=p2[:, kc], in0=vt[:, kc],
                                                in1=he[:, kc], op=MULT)
                        dots = sbs.tile([128, BLK], F32, tag="dots")
                        nc.vector.tensor_reduce(
                            out=dots[:, :k],
                            in_=p2[:, kc].rearrange("p (k d) -> p k d", d=D),
                            axis=mybir.AxisListType.X, op=mybir.AluOpType.add)
                        e1 = sbs.tile([128, BLK], F32, tag="e1")
                        nc.scalar.activation(e1[:, :k], dots[:, :k], EXP)
                        wsc = sbs.tile([128, BLK], BF16, tag="w8")
                        nc.scalar.activation(wsc[:, :k], e1[:, :k], EXP)
                    else:
                        mmt = sb3.tile([128, BLK * 512], FP8, tag="mm")
                        nc.sync.dma_start(mmt[:, 0:k * 128],
                                          t_imm.ap()[:, c0 * 128:(c0 + k) * 128])
                        io = ioffs[bi]
                        wsc = idx_i[:, io + k * 8:io + k * 9].bitcast(BF16)

                    tew = sb3.tile([128, BLK * 128], BF16, tag="tew")
                    nc.vector.tensor_tensor(
                        out=tew[:, kc].rearrange("p (k d) -> p k d", d=D),
                        in0=te[:, kc].rearrange("p (k d) -> p k d", d=D),
                        in1=wsc[:, :k].rearrange("p (k o) -> p k o", o=1)
                            .to_broadcast([128, k, D]),
                        op=MULT)

                    # interleave window order so consecutive matmuls alternate
                    # PSUM banks (w%BLK*512B: windows 0-3 bank A, 4-7 bank B)
                    order = sorted(range(k), key=lambda j: (entries[j][1] % BLK) % 4 * 2
                                   + (entries[j][1] % BLK) // 4)
                    for j in order:
                        (cid, w, kk, first, last) = entries[j]
                        sl = slice(j * 128, (j + 1) * 128)
                        aggv = aggt[:, (w % BLK) * D:(w % BLK + 1) * D]
                        m2sl = (mmt[:, j * 256 + 128:(j + 1) * 256] if is_hop
                                else mmt[:, j * 128:(j + 1) * 128])
                        nc.tensor.matmul(aggv, m2sl, tew[:, sl],
                                         start=False, stop=bool(last),
                                         skip_group_check=True)
                    for j, (cid, w, kk, first, last) in enumerate(entries):
                        if not last:
                            continue
                        aggv = aggt[:, (w % BLK) * D:(w % BLK + 1) * D]
                        if not pend:
                            ent8 = nbp.tile([128, NB * D], F32, tag="ent8")
                            ssqs = nsc.tile([128, NB], F32, tag="ssqs")
                        i8 = w % BLK
                        dst = ent8[:, i8 * D:(i8 + 1) * D]
                        nc.scalar.copy(dst, aggv)
                        scr = nsc.tile([128, D], F32, tag="sqscr")
                        nc.scalar.activation(scr[:], dst, SQ,
                                             accum_out=ssqs[:, i8:i8 + 1])
                        pend.append(w)
                        if len(pend) == NB:
                            flush(aggt)
                flush(aggt)

            for h in range(n_hops + 1):
                stage(h)
    nc.compile()
    return nc


_CACHE = {}


def kernel(user_emb, item_emb, edge_index, edge_type, inter_edge, inter_edge_w,
           relation_emb, n_hops, _trace=False):
    n_hops = int(n_hops)
    item_emb = np.asarray(item_emb, dtype=np.float32)
    relation_emb = np.asarray(relation_emb, dtype=np.float32)

    per_core, struct, perm_ent, perm_usr = _preprocess(
        item_emb, edge_index, edge_type, inter_edge, inter_edge_w, relation_emb)
    key = (struct, n_hops)
    if key not in _CACHE:
        _CACHE[key] = _build_program(struct, n_hops)
    nc = _CACHE[key]

    in_maps = []
    for c in range(N_CORES):
        pc = per_core[c]
        in_maps.append({
            "ent_slice": pc["ent_slice"],
            "mm": pc["mm"],
            "h0te": pc["h0te"], "h1idx": pc["h1idx"],
            "imm": pc["imm"], "iidx": pc["iidx"],
        })
    import os
    kw = {}
    if _trace and os.environ.get("KERNEL_NTFF_DIR"):
        os.makedirs(os.environ["KERNEL_NTFF_DIR"], exist_ok=True)
        kw["tmpdir"] = os.environ["KERNEL_NTFF_DIR"]
    res = bass_utils.run_bass_kernel_spmd(
        nc, in_maps, core_ids=list(range(N_CORES)), trace=_trace, **kw,
    )
    # outputs are [128, WPC*D] window-major: row l=p*WPC+w -> [p, w*D:(w+1)*D]
    ent_p = np.concatenate(
        [res.results[c]["ent_out"].reshape(WB, D) for c in range(N_CORES)], 0)
    usr_p = np.concatenate(
        [res.results[c]["user_out"].reshape(WB, D) for c in range(N_CORES)], 0)
    vmask = perm_ent >= 0
    ent = np.empty((N_NODES, D), np.float32)
    ent[perm_ent[vmask]] = ent_p[vmask]
    usr = np.empty((N_USERS, D), np.float32)
    vmask_u = perm_usr >= 0
    usr[perm_usr[vmask_u]] = usr_p[vmask_u]
    if _trace:
        kernel._last_exec_ns = res.exec_time_ns
        kernel._last_res = res
    return usr, ent
